# revision 40
# baseline (speedup 1.0000x reference)
"""Trainium2 Bass kernel for nn_Encoder_base (5x ChebConv GNN + pool + MLP).

Distribution over 8 NeuronCores (all matmuls fp16, fp32 PSUM):
  - level 0: the two props the einsum needs (Tx1[l0] = S0[l0]@X0 and
    p2t = S0[l0]@S0@X0) are composed on the HOST into single operators on
    the input X0 (2-hop edge expansion M0 = S0[l0]*S0). Edge-major X0 rows
    are pregathered host-side -> the props are pure streaming selection
    matmuls: zero indirect DMA, zero full-graph AllGather.
  - level 1: stacked dense operator T = [P_l1; S1[l1]; M1=S1[l1]*S1]
    (3072 x 4096) applied to z1, dest-sharded (128 l1-positions/core, all
    1024 batch-features wide); einsum is dest-sharded too. Comm: one
    window-chunked z1 AllGather (overlapped with the l0 pipeline) + one
    z2 AllToAll.
  - levels 2-3: batch-sharded (4 batches/core), dense-S matmuls,
    block-diagonal channel mixes in feature-major layout.
  - MLP: output-feature sharded (512 cols of W6/7/8, 512 rows of W9 per
    core); activations [128k,32] are the stationary lhsT, W streams as rhs;
    BatchNorm per-feature after a PE transpose; activations AllGathered.
"""
import numpy as np
import concourse.bass as bass
import concourse.bacc as bacc
import concourse.tile as tile
from concourse import mybir, bass_utils

F32 = mybir.dt.float32
F16 = mybir.dt.float16
I16 = mybir.dt.int16
AF = mybir.ActivationFunctionType
ALU = mybir.AluOpType
AX = mybir.AxisListType
RG = [list(range(8))]
NCORES = 8
N0, N1, N2, N3 = 16384, 4096, 1024, 128
EPS = 1e-5
H16 = np.float16

_CACHE = {}


# ---------------------------------------------------------------- host prep
def _prep_prop(row, col, we, n_dest, n_shard):
    """Sorted-by-dest edges -> 128-dest windows, 128-edge chunks, padded so
    chunk counts per window match across shards (one SPMD program).
    Emits per-chunk selection matrices sel[chunk, edge_local, dst_local]."""
    window = 128
    order = np.argsort(row, kind="stable")
    row, col, we = row[order], col[order], we[order]
    per = n_dest // n_shard
    nwin = per // window
    counts = np.zeros((n_shard, nwin), np.int64)
    lists = {}
    for s in range(n_shard):
        lo = s * per
        for wi in range(nwin):
            wlo = lo + wi * window
            a = np.searchsorted(row, wlo, side="left")
            b = np.searchsorted(row, wlo + window, side="left")
            lists[(s, wi)] = (row[a:b] - wlo, col[a:b], we[a:b])
            counts[s, wi] = (b - a + 127) // 128
    ncw = np.maximum(counts.max(axis=0), 1)
    C = int(ncw.sum())
    src = np.zeros((n_shard, C, 128), np.int64)
    dst = np.full((n_shard, C, 128), 200.0, np.float32)
    wea = np.zeros((n_shard, C, 128), np.float32)
    for s in range(n_shard):
        base = 0
        for wi in range(nwin):
            dl, cl, wl = lists[(s, wi)]
            n = len(dl)
            k = int(ncw[wi])
            src[s, base:base + k].reshape(-1)[:n] = cl
            ch = base + np.arange(n) // 128
            ep = np.arange(n) % 128
            dst[s, ch, ep] = dl
            wea[s, ch, ep] = wl
            base += k
    return [int(x) for x in ncw], src, dst, wea


def _edge_we(e, n):
    row, col = np.asarray(e[0], np.int64), np.asarray(e[1], np.int64)
    deg = np.bincount(row, minlength=n).astype(np.float32)
    dis = np.where(deg > 0, 1.0 / np.sqrt(np.maximum(deg, 1.0)), 0.0).astype(np.float32)
    return row, col, -(dis[row] * dis[col]).astype(np.float32)


def _sub_edges(row, col, we, pool_idx):
    order = np.argsort(row, kind="stable")
    row, col, we = row[order], col[order], we[order]
    starts = np.searchsorted(row, pool_idx, side="left")
    ends = np.searchsorted(row, pool_idx, side="right")
    nr, ncl, nw = [], [], []
    for i in range(len(pool_idx)):
        s, e = starts[i], ends[i]
        if e > s:
            nr.append(np.full(e - s, i, np.int64))
            ncl.append(col[s:e])
            nw.append(we[s:e])
    return np.concatenate(nr), np.concatenate(ncl), np.concatenate(nw)


def _twohop(ri, ci, wi, row, col, we, n):
    """(i,j,w1) sub-edges composed with full edges (j->k,w2): (i,k,w1*w2)."""
    order = np.argsort(row, kind="stable")
    rs, cs, ws = row[order], col[order], we[order]
    starts = np.searchsorted(rs, np.arange(n), side="left")
    ends = np.searchsorted(rs, np.arange(n), side="right")
    cnt = (ends - starts)[ci]
    I = np.repeat(ri, cnt)
    W1 = np.repeat(wi, cnt)
    base = np.repeat(starts[ci], cnt)
    within = np.arange(cnt.sum()) - np.repeat(np.cumsum(cnt) - cnt, cnt)
    offs = base + within
    return I, cs[offs], W1 * ws[offs]


def _dense_s(row, col, we, n, m):
    s = np.zeros((n, m), np.float32)
    np.add.at(s, (row, col), we)
    return s


def _tile_w(w, pack):
    """[K, M] -> [K//(128*pack) * 128, pack*M]: pack K-blocks side by side."""
    k, m = w.shape
    nb = k // 128
    t = w.reshape(nb // pack, pack, 128, m).transpose(0, 2, 1, 3)
    return np.ascontiguousarray(t.reshape((nb // pack) * 128, pack * m))


def _idx_tile(flat):
    """flat int idx list -> [128, len//16] int16 (16-part wrap, x8 replicas)."""
    return np.ascontiguousarray(
        np.tile(flat.astype(np.int16).reshape(-1, 16).T, (8, 1)))


def _chunk_tile(arr3):
    """[C, 128, W] -> [128, C*W] (chunk c at cols c*W..)."""
    C, _, W = arr3.shape
    return np.ascontiguousarray(
        arr3.transpose(1, 0, 2).reshape(128, C * W)).astype(H16)


def _host_prep(inputs):
    d = {k: np.asarray(v) for k, v in inputs.items()}
    x = d["x"].astype(np.float32)
    l0 = np.asarray(d["l0"], np.int64)
    l1 = np.asarray(d["l1"], np.int64)
    l2 = np.asarray(d["l2"], np.int64)

    X0 = np.ascontiguousarray(x.transpose(1, 0, 2).reshape(N0, 96))
    X0p = np.zeros((N0, 128), np.float32)
    X0p[:, :96] = X0
    X0l0T = np.ascontiguousarray(X0[l0].T)  # [96, 4096]

    # level-0 operators on X0: a = S0[l0] (tap1), m = S0[l0]@S0 (tap2)
    r0, c0, w0 = _edge_we(d["e0"], N0)
    r0s, c0s, w0s = _sub_edges(r0, c0, w0, l0)
    ncw_a, src_a, dst_a, we_a = _prep_prop(r0s, c0s, w0s, N1, NCORES)
    mI, mK, mW = _twohop(r0s, c0s, w0s, r0, c0, w0, N0)
    ncw_m, src_m, dst_m, we_m = _prep_prop(mI, mK, mW, N1, NCORES)

    # level-1 stacked operator T = [P_l1; S1[l1]; M1]
    r1, c1, w1 = _edge_we(d["e1"], N1)
    S1 = _dense_s(r1, c1, w1, N1, N1)
    r1s, c1s, w1s = _sub_edges(r1, c1, w1, l1)
    S1l1 = _dense_s(r1s, c1s, w1s, N2, N1)    # [1024, 4096]
    M1 = S1l1 @ S1                            # [1024, 4096]
    P_l1 = np.zeros((N2, N1), np.float32)
    P_l1[np.arange(N2), l1] = 1.0
    Tblocks = [P_l1, S1l1, M1]

    r2, c2, w2 = _edge_we(d["e2"], N2)
    S2 = _dense_s(r2, c2, w2, N2, N2)
    S2T = _tile_w(np.ascontiguousarray(S2.T), 8).astype(H16)       # [128, 8192]
    S2l2T = _tile_w(np.ascontiguousarray(S2[l2].T), 8).astype(H16)  # [128, 1024]
    P_l2 = np.zeros((N2, 128), np.float32)
    P_l2[l2, np.arange(128)] = 1.0
    P_l2 = _tile_w(P_l2, 8).astype(H16)                             # [128, 1024]

    r3, c3, w3 = _edge_we(d["e3"], N3)
    S3T = np.ascontiguousarray(_dense_s(r3, c3, w3, N3, N3).T).astype(H16)

    def wmod(W):
        return W[0] - W[2], W[1], 2.0 * W[2]

    Wm1 = wmod(d["Wc1"].astype(np.float32))
    Wm = [wmod(d[f"Wc{i}"].astype(np.float32)) for i in (2, 3, 4, 5)]
    eye4 = np.eye(4, dtype=np.float32)

    per_core = []
    for k in range(NCORES):
        m = {}
        m["identbf"] = np.eye(128, dtype=np.float32).astype(H16)
        m["iota"] = np.tile(np.arange(128, dtype=np.float32), (128, 1))
        m["epsv"] = np.full((128, 1), EPS, np.float32)
        m["X0l0Tw"] = np.ascontiguousarray(
            X0l0T[:, 512 * k:512 * (k + 1)]).astype(H16)
        m["Xg_a"] = _chunk_tile(X0p[src_a[k]].astype(H16))
        m["a_dst"] = np.ascontiguousarray(dst_a[k].T)
        m["a_we"] = np.ascontiguousarray(we_a[k].T)
        m["Xg_m"] = _chunk_tile(X0p[src_m[k]].astype(H16))
        m["m_dst"] = np.ascontiguousarray(dst_m[k].T)
        m["m_we"] = np.ascontiguousarray(we_m[k].T)
        # stacked-T lhsT chunks: block b, k-chunk kk at cols (b*32+kk)*128
        tt = np.zeros((128, 96 * 128), np.float32)
        for b, blk in enumerate(Tblocks):
            bt = blk[128 * k:128 * (k + 1), :].T  # [4096, 128]
            for kk in range(32):
                tt[:, (b * 32 + kk) * 128:(b * 32 + kk + 1) * 128] = \
                    bt[128 * kk:128 * (kk + 1), :]
        m["Tt"] = tt.astype(H16)
        m["S2T"] = S2T
        m["S2l2T"] = S2l2T
        m["P_l2"] = P_l2
        m["S3T"] = S3T
        for g in range(8):
            for t in range(3):
                bw = np.zeros((96, 128), np.float32)
                for j in range(4):
                    bg = 4 * g + j
                    bw[3 * bg:3 * bg + 3, 32 * j:32 * j + 32] = Wm1[t]
                m[f"bigw0_{g}_{t}"] = bw.astype(H16)
        for lev in range(4):
            for t in range(3):
                m[f"bigw{lev + 1}_{t}"] = np.kron(eye4, Wm[lev][t]).astype(H16)
        for lev, nm in ((1, "b1"), (2, "b2"), (3, "b3"), (4, "b4"), (5, "b5")):
            m[f"bias{lev}"] = np.tile(d[nm].astype(np.float32), 4).reshape(128, 1)
        for li in (6, 7, 8):
            W = d[f"W{li}"].astype(np.float32)[:, 512 * k:512 * k + 512]
            m[f"w{li}"] = np.ascontiguousarray(
                W.reshape(32, 128, 512).transpose(1, 0, 2).reshape(128, 32 * 512)
            ).astype(H16)
            m[f"g{li}"] = np.ascontiguousarray(
                d[f"g{li}"].astype(np.float32)[512 * k:512 * k + 512].reshape(4, 128).T)
            m[f"be{li}"] = np.ascontiguousarray(
                d[f"be{li}"].astype(np.float32)[512 * k:512 * k + 512].reshape(4, 128).T)
        W9 = d["W9"].astype(np.float32)[512 * k:512 * k + 512]  # [512, 128]
        m["w9"] = np.ascontiguousarray(
            W9.reshape(4, 128, 128).transpose(1, 0, 2).reshape(128, 512)).astype(H16)
        per_core.append(m)

    meta = {"a": ncw_a, "m": ncw_m}
    return per_core, meta


# ---------------------------------------------------------------- device program
def _build_nc(meta, shapes):
    nc = bacc.Bacc("TRN2", target_bir_lowering=False, debug=False, num_devices=NCORES)
    ein = {}
    for name, arr in shapes.items():
        dt = {np.dtype(np.int16): I16, np.dtype(H16): F16,
              np.dtype(np.float32): F32}[arr.dtype]
        ein[name] = nc.dram_tensor(name, list(arr.shape), dt, kind="ExternalInput")
    out_mu = nc.dram_tensor("mu", [128, 32], F32, kind="ExternalOutput")

    warm_all = nc.dram_tensor("warm_all", [8, 8], F32, addr_space="Shared")
    warm_loc = nc.dram_tensor("warm_loc", [1, 8], F32)
    z1c = [nc.dram_tensor(f"z1c_{i}", [128, 1024], F16) for i in range(4)]
    z1ag = [nc.dram_tensor(f"z1ag_{i}", [1024, 1024], F16, addr_space="Shared")
            for i in range(4)]
    z2T_loc = nc.dram_tensor("z2T_loc", [1024, 128], F16)
    z2a2a = nc.dram_tensor("z2a2a", [1024, 128], F16)
    x_loc = nc.dram_tensor("x_loc", [4, 4096], F16)
    x_all = nc.dram_tensor("x_all", [32, 4096], F16, addr_space="Shared")
    h6_loc = nc.dram_tensor("h6_loc", [128, 128], F16)
    h6_all = nc.dram_tensor("h6_all", [1024, 128], F16, addr_space="Shared")
    h7_loc = nc.dram_tensor("h7_loc", [128, 128], F16)
    h7_all = nc.dram_tensor("h7_all", [1024, 128], F16, addr_space="Shared")
    p9_loc = nc.dram_tensor("p9_loc", [32, 128], F32)
    p9_red = nc.dram_tensor("p9_red", [32, 128], F32, addr_space="Shared")

    def ag(loc_ap, all_ap):
        nc.gpsimd.collective_compute(
            "AllGather", ALU.bypass, replica_groups=RG,
            ins=[loc_ap.opt()], outs=[all_ap.opt()])

    with tile.TileContext(nc) as tc:
        with (
            tc.tile_pool(name="const", bufs=1) as cpool,
            tc.tile_pool(name="big", bufs=1) as bigpool,
            tc.tile_pool(name="work", bufs=3) as wpool,
            tc.tile_pool(name="wload", bufs=2) as wlpool,
            tc.tile_pool(name="psA", bufs=4, space="PSUM") as ppool,
            tc.tile_pool(name="psT", bufs=2, space="PSUM") as tpool,
            tc.tile_pool(name="psB", bufs=1, space="PSUM") as apool,
        ):
            ident = cpool.tile([128, 128], F16, tag="identbf", name="identbf")
            nc.sync.dma_start(out=ident[:], in_=ein["identbf"][:, :])
            iota_t = cpool.tile([128, 128], F32, tag="iota", name="iota")
            nc.sync.dma_start(out=iota_t[:], in_=ein["iota"][:, :])
            eps_t = cpool.tile([128, 1], F32, tag="epsv", name="epsv")
            nc.sync.dma_start(out=eps_t[:], in_=ein["epsv"][:, :])

            def load_const(name, dt=F16):
                t = cpool.tile(list(shapes[name].shape), dt, tag=name)
                nc.sync.dma_start(out=t[:], in_=ein[name][:, :])
                return t

            GRP = 16

            def grp_load(pref, g0, gc, tag, eng=None, grp=None):
                sl = wpool.tile([128, (grp or GRP) * 128], F16, tag=tag,
                                name=tag, bufs=2)
                (eng or nc.sync).dma_start(out=sl[:, :gc * 128],
                                           in_=ein[pref][:, g0 * 128:(g0 + gc) * 128])
                return sl

            def transp(src_ap, dst_ap):
                p, f = src_ap.shape
                ps = tpool.tile([128, 128], F16, tag="tp", name="tp")
                nc.tensor.transpose(out=ps[:f, :p], in_=src_ap, identity=ident[:p, :p])
                nc.scalar.activation(out=dst_ap, in_=ps[:f, :p], func=AF.Copy)

            def einsum_win(bigw, taps, Din, width, out_ap, func, bias_ap):
                ps = ppool.tile([128, 512], F32, tag="ps", name="ps")
                for t in range(3):
                    nc.tensor.matmul(out=ps[:, :width], lhsT=bigw[t][:Din, :],
                                     rhs=taps[t], start=(t == 0), stop=(t == 2))
                f2 = AF.Identity if func == AF.Copy else func
                nc.scalar.activation(out=out_ap, in_=ps[:, :width], func=f2, bias=bias_ap)

            # warm up the CC ring while level-0 computes
            with nc.named_scope("warmup"):
                ag(warm_loc.ap(), warm_all.ap())

            # ====== LEVEL 0: per-window pipeline of props -> einsum -> AG ====
            # tap1 = S0[l0]@X0, tap2 = (S0[l0]@S0)@X0, then the channel-mix
            # einsum for window wi immediately, then AllGather that window.
            with nc.named_scope("l0"):
                Ca, Cm = sum(meta["a"]), sum(meta["m"])
                cura = {"g0": -1}
                curm = {"g0": -1}

                def get_grp(cur, cc, C, xg, xtag, grp):
                    g0 = (cc // grp) * grp
                    if g0 != cur["g0"]:
                        gc = min(grp, C - g0)
                        cur["g0"] = g0
                        cur["x"] = grp_load(xg, g0, gc, xtag, eng=nc.scalar, grp=grp)
                    return cur, (cc - cur["g0"]) * 128

                def mk_sel(dw_t, cc):
                    sel = wpool.tile([128, 128], F16, tag="sel", name="sel", bufs=4)
                    nc.vector.tensor_scalar(
                        out=sel[:], in0=iota_t[:], scalar1=dw_t[0][:, cc:cc + 1],
                        scalar2=dw_t[1][:, cc:cc + 1], op0=ALU.is_equal, op1=ALU.mult)
                    return sel

                adw = [load_const("a_dst", F32), load_const("a_we", F32)]
                mdw = [load_const("m_dst", F32), load_const("m_we", F32)]
                tt = cpool.tile([128, 96 * 128], F16, tag="Tt", name="Tt")
                nc.sync.dma_start(out=tt[:, :6144], in_=ein["Tt"][:, :6144])
                nc.sync.dma_start(out=tt[:, 6144:], in_=ein["Tt"][:, 6144:])
                accs = [bigpool.tile([128, 1024], F16, tag=f"accT{b}", name=f"accT{b}")
                        for b in range(3)]

                def t_part(part):
                    for h in range(2):
                        z1sc = wpool.tile([128, 4096], F16, tag="z1sc",
                                          name="z1sc", bufs=2)
                        nc.sync.dma_start(
                            out=z1sc[:].rearrange("p (q d) -> p q d", d=1024),
                            in_=z1ag[part].ap()[512 * h:512 * (h + 1)]
                                .rearrange("(q p) d -> p q d", p=128))
                        for b in range(3):
                            psa = ppool.tile([128, 512], F32, tag="ps", name="ps")
                            psb = ppool.tile([128, 512], F32, tag="ps", name="ps")
                            for q in range(4):
                                kk = 4 * (4 * h + q) + part
                                lh = tt[:, (b * 32 + kk) * 128:(b * 32 + kk + 1) * 128]
                                nc.tensor.matmul(out=psa[:, :512], lhsT=lh,
                                                 rhs=z1sc[:, q * 1024:q * 1024 + 512],
                                                 start=(q == 0), stop=(q == 3))
                                nc.tensor.matmul(out=psb[:, :512], lhsT=lh,
                                                 rhs=z1sc[:, q * 1024 + 512:(q + 1) * 1024],
                                                 start=(q == 0), stop=(q == 3))
                            if part == 0 and h == 0:
                                nc.scalar.activation(out=accs[b][:, :512],
                                                     in_=psa[:, :512], func=AF.Copy)
                                nc.scalar.activation(out=accs[b][:, 512:],
                                                     in_=psb[:, :512], func=AF.Copy)
                            else:
                                nc.vector.tensor_add(accs[b][:, :512], accs[b][:, :512],
                                                     psa[:, :512])
                                nc.vector.tensor_add(accs[b][:, 512:], accs[b][:, 512:],
                                                     psb[:, :512])

                bias1 = load_const("bias1", F32)
                x0w = cpool.tile([96, 512], F16, tag="X0l0Tw", name="X0l0Tw")
                nc.sync.dma_start(out=x0w[:], in_=ein["X0l0Tw"][:, :])
                bw0g = [[load_const(f"bigw0_{g}_{t}") for t in range(3)]
                        for g in range(8)]
                tap1_sb = bigpool.tile([96, 512], F16, tag="tap1_sb", name="tap1_sb")
                tap2_sb = bigpool.tile([96, 512], F16, tag="tap2_sb", name="tap2_sb")
                ba, bm = 0, 0
                for wi in range(4):
                    psa = ppool.tile([128, 512], F32, tag="ps", name="ps")
                    for c in range(meta["a"][wi]):
                        g, lo = get_grp(cura, ba + c, Ca, "Xg_a", "xga", 8)
                        nc.tensor.matmul(out=psa[:96, :128], lhsT=g["x"][:, lo:lo + 96],
                                         rhs=mk_sel(adw, ba + c)[:],
                                         start=(c == 0), stop=(c == meta["a"][wi] - 1))
                    nc.scalar.activation(out=tap1_sb[:, wi * 128:(wi + 1) * 128],
                                         in_=psa[:96, :128], func=AF.Copy)
                    ba += meta["a"][wi]
                    psm = ppool.tile([128, 512], F32, tag="ps", name="ps")
                    for c in range(meta["m"][wi]):
                        g, lo = get_grp(curm, bm + c, Cm, "Xg_m", "xgm", 16)
                        nc.tensor.matmul(out=psm[:96, :128], lhsT=g["x"][:, lo:lo + 96],
                                         rhs=mk_sel(mdw, bm + c)[:],
                                         start=(c == 0), stop=(c == meta["m"][wi] - 1))
                    nc.scalar.activation(out=tap2_sb[:, wi * 128:(wi + 1) * 128],
                                         in_=psm[:96, :128], func=AF.Copy)
                    bm += meta["m"][wi]
                    for g in range(8):
                        z1gT = wpool.tile([128, 128], F16, tag="z1Tw", name="z1Tw")
                        einsum_win(bw0g[g],
                                   [x0w[:, wi * 128:(wi + 1) * 128],
                                    tap1_sb[:96, wi * 128:(wi + 1) * 128],
                                    tap2_sb[:96, wi * 128:(wi + 1) * 128]],
                                   96, 128, z1gT[:], AF.Copy, bias1[:, 0:1])
                        t = wpool.tile([128, 128], F16, tag="z1nc", name="z1nc")
                        transp(z1gT[:], t[:])
                        nc.sync.dma_start(
                            out=z1c[wi][:, 128 * g:128 * (g + 1)], in_=t[:])
                    with nc.named_scope(f"agz1_{wi}"):
                        ag(z1c[wi].ap(), z1ag[wi].ap())
                with nc.named_scope("l1_T"):
                    for part in range(4):
                        t_part(part)

            # ============ LEVEL 1 einsum (dest-sharded) -> z2T ===============
            with nc.named_scope("l1_einsum"):
                bw1 = [load_const(f"bigw1_{t}") for t in range(3)]
                bias2 = load_const("bias2", F32)
                tapTs = []
                for b in range(3):
                    tapT = bigpool.tile([128, 1024], F16, tag=f"tapT{b}", name=f"tapT{b}")
                    for f in range(8):
                        transp(accs[b][:, 128 * f:128 * (f + 1)],
                               tapT[:, 128 * f:128 * (f + 1)])
                    tapTs.append(tapT)
                for fg in range(8):
                    z2fg = wpool.tile([128, 128], F16, tag="z2fg", name="z2fg")
                    einsum_win(bw1, [tapTs[0][:, 128 * fg:128 * (fg + 1)],
                                     tapTs[1][:, 128 * fg:128 * (fg + 1)],
                                     tapTs[2][:, 128 * fg:128 * (fg + 1)]],
                               128, 128, z2fg[:], AF.Tanh, bias2[:, 0:1])
                    nc.sync.dma_start(out=z2T_loc[128 * fg:128 * (fg + 1), :],
                                      in_=z2fg[:])
            with nc.named_scope("a2a_z2"):
                nc.gpsimd.collective_compute(
                    "AllToAll", ALU.bypass, replica_groups=RG,
                    ins=[z2T_loc.ap().opt()], outs=[z2a2a.ap().opt()])

            # ====== z2n assembly (batch-sharded node-major) ==================
            z2n = bigpool.tile([128, 8 * 128], F16, tag="z2n", name="z2n")
            with nc.named_scope("z2n_asm"):
                zb = wpool.tile([128, 1024], F16, tag="zb", name="zb", bufs=1)
                for ci in range(8):
                    eng = nc.scalar if ci % 2 else nc.sync
                    eng.dma_start(out=zb[:, ci * 128:(ci + 1) * 128],
                                  in_=z2a2a[128 * ci:128 * (ci + 1), :])
                for ci in range(8):
                    transp(zb[:, ci * 128:(ci + 1) * 128],
                           z2n[:, ci * 128:(ci + 1) * 128])

            # ================= LEVEL 2 (dense) =================
            with nc.named_scope("l2"):
                t1_l2 = bigpool.tile([128, 8 * 128], F16, tag="t1_l2", name="t1_l2")
                for half in range(2):
                    s2t = wlpool.tile([128, 4096], F16, tag="wld", name="wld")
                    nc.sync.dma_start(out=s2t[:], in_=ein["S2T"][:, 4096 * half:4096 * (half + 1)])
                    for dc in range(8):
                        ps = ppool.tile([128, 512], F32, tag="ps", name="ps")
                        for kk in range(4):
                            kc = half * 4 + kk
                            nc.tensor.matmul(
                                out=ps[:, :128],
                                lhsT=s2t[:, kk * 1024 + dc * 128: kk * 1024 + dc * 128 + 128],
                                rhs=z2n[:, kc * 128:(kc + 1) * 128],
                                start=(kk == 0), stop=(kk == 3))
                        if half == 0:
                            nc.scalar.activation(out=t1_l2[:, dc * 128:(dc + 1) * 128],
                                                 in_=ps[:, :128], func=AF.Copy)
                        else:
                            nc.vector.tensor_add(t1_l2[:, dc * 128:(dc + 1) * 128],
                                                 t1_l2[:, dc * 128:(dc + 1) * 128],
                                                 ps[:, :128])
                s2l2 = cpool.tile([128, 1024], F16, tag="s2l2", name="s2l2")
                nc.sync.dma_start(out=s2l2[:], in_=ein["S2l2T"][:, :])
                ps = ppool.tile([128, 512], F32, tag="ps", name="ps")
                for kc in range(8):
                    nc.tensor.matmul(out=ps[:, :128], lhsT=s2l2[:, kc * 128:(kc + 1) * 128],
                                     rhs=t1_l2[:, kc * 128:(kc + 1) * 128],
                                     start=(kc == 0), stop=(kc == 7))
                p2n_l2 = wpool.tile([128, 128], F16, tag="p2n_l2", name="p2n_l2")
                nc.scalar.activation(out=p2n_l2[:], in_=ps[:, :128], func=AF.Copy)
                pl2 = cpool.tile([128, 1024], F16, tag="pl2", name="pl2")
                nc.sync.dma_start(out=pl2[:], in_=ein["P_l2"][:, :])
                z2l2T = wpool.tile([128, 128], F16, tag="z2l2T", name="z2l2T")
                psg = ppool.tile([128, 512], F32, tag="ps", name="ps")
                for kc in range(8):
                    nc.tensor.matmul(out=psg[:, :128], lhsT=z2n[:, kc * 128:(kc + 1) * 128],
                                     rhs=pl2[:, kc * 128:(kc + 1) * 128],
                                     start=(kc == 0), stop=(kc == 7))
                nc.scalar.activation(out=z2l2T[:], in_=psg[:, :128], func=AF.Copy)
                t1l2T = wpool.tile([128, 128], F16, tag="t1l2T", name="t1l2T")
                psg2 = ppool.tile([128, 512], F32, tag="ps", name="ps")
                for kc in range(8):
                    nc.tensor.matmul(out=psg2[:, :128], lhsT=t1_l2[:, kc * 128:(kc + 1) * 128],
                                     rhs=pl2[:, kc * 128:(kc + 1) * 128],
                                     start=(kc == 0), stop=(kc == 7))
                nc.scalar.activation(out=t1l2T[:], in_=psg2[:, :128], func=AF.Copy)
                p2l2T = wpool.tile([128, 128], F16, tag="p2l2T", name="p2l2T")
                transp(p2n_l2[:], p2l2T[:])
                bw2 = [load_const(f"bigw2_{t}") for t in range(3)]
                bias3 = load_const("bias3", F32)
                z3T = wpool.tile([128, 128], F16, tag="z3T", name="z3T")
                einsum_win(bw2, [z2l2T[:], t1l2T[:], p2l2T[:]], 128, 128,
                           z3T[:], AF.Tanh, bias3[:, 0:1])
                z3n = wpool.tile([128, 128], F16, tag="z3n", name="z3n")
                transp(z3T[:], z3n[:])

            # ================= LEVEL 3 =================
            with nc.named_scope("l3"):
                s3t = cpool.tile([128, 128], F16, tag="s3t", name="s3t")
                nc.sync.dma_start(out=s3t[:], in_=ein["S3T"][:, :])
                bias4 = load_const("bias4", F32)
                bias5 = load_const("bias5", F32)

                def conv_l3(zn, zT, bw_pref, bias_t, func, keep):
                    t1T = wpool.tile([128, 128], F16, tag=keep + "t1T", name=keep + "t1T")
                    ps = ppool.tile([128, 512], F32, tag="ps", name="ps")
                    nc.tensor.matmul(out=ps[:, :128], lhsT=zn, rhs=s3t[:], start=True, stop=True)
                    nc.scalar.activation(out=t1T[:], in_=ps[:, :128], func=AF.Copy)
                    t1n_ = wpool.tile([128, 128], F16, tag=keep + "t1n", name=keep + "t1n")
                    transp(t1T[:], t1n_[:])
                    p2T_ = wpool.tile([128, 128], F16, tag=keep + "p2T", name=keep + "p2T")
                    ps2 = ppool.tile([128, 512], F32, tag="ps", name="ps")
                    nc.tensor.matmul(out=ps2[:, :128], lhsT=t1n_[:], rhs=s3t[:], start=True, stop=True)
                    nc.scalar.activation(out=p2T_[:], in_=ps2[:, :128], func=AF.Copy)
                    bw = [load_const(f"{bw_pref}_{t}") for t in range(3)]
                    outT = wpool.tile([128, 128], F16, tag=keep + "oT", name=keep + "oT")
                    einsum_win(bw, [zT, t1T[:], p2T_[:]], 128, 128, outT[:], func, bias_t[:, 0:1])
                    outn = wpool.tile([128, 128], F16, tag=keep + "on", name=keep + "on")
                    transp(outT[:], outn[:])
                    return outn, outT

                z4n, z4T = conv_l3(z3n[:], z3T[:], "bigw3", bias4, AF.Tanh, "c4")
                o5n, o5T = conv_l3(z4n[:], z4T[:], "bigw4", bias5, AF.Copy, "c5")

            # ================= MLP input assembly =================
            with nc.named_scope("mlp_in"):
                nc.sync.dma_start(
                    out=x_loc.ap().rearrange("b (n c) -> n b c", c=32),
                    in_=o5n[:].rearrange("n (b c) -> n b c", c=32))
                ag(x_loc.ap(), x_all.ap())
                xT_sb = bigpool.tile([32, 4096], F16, tag="xT_sb", name="xT_sb")
                nc.sync.dma_start(out=xT_sb[:], in_=x_all[:, :])
                act6 = bigpool.tile([128, 1024], F16, tag="act6", name="act6")
                for i in range(32):
                    transp(xT_sb[:, 128 * i:128 * (i + 1)], act6[:, 32 * i:32 * i + 32])

            # ================= MLP =================
            def mlp_layer(li, act_sb, out_sb):
                g_t = load_const(f"g{li}", F32)
                be_t = load_const(f"be{li}", F32)
                wt = wlpool.tile([128, 32 * 512], F16, tag="wld", name="wld")
                nc.sync.dma_start(out=wt[:, :8192], in_=ein[f"w{li}"][:, :8192])
                nc.sync.dma_start(out=wt[:, 8192:], in_=ein[f"w{li}"][:, 8192:])
                acc = apool.tile([128, 512], F32, tag="acc", name="acc")
                for k2 in range(32):
                    nc.tensor.matmul(out=acc[:32, :], lhsT=act_sb[:, 32 * k2:32 * k2 + 32],
                                     rhs=wt[:, 512 * k2:512 * (k2 + 1)],
                                     start=(k2 == 0), stop=(k2 == 31))
                hb = wpool.tile([32, 512], F16, tag="hb", name="hb")
                nc.scalar.activation(out=hb[:], in_=acc[:32, :], func=AF.Copy)
                for c in range(4):
                    hc = wpool.tile([128, 32], F16, tag="hc", name="hc", bufs=4)
                    transp(hb[:, 128 * c:128 * (c + 1)], hc[:])
                    st6 = wpool.tile([128, 6], F32, tag="b_st6", name="b_st6")
                    nc.vector.bn_stats(out=st6[:], in_=hc[:])
                    mv = wpool.tile([128, 2], F32, tag="b_mv", name="b_mv")
                    nc.vector.bn_aggr(out=mv[:], in_=st6[:])
                    sd = wpool.tile([128, 1], F32, tag="b_sd", name="b_sd")
                    nc.scalar.activation(out=sd[:], in_=mv[:, 1:2], func=AF.Sqrt,
                                         bias=eps_t[:, 0:1])
                    rs = wpool.tile([128, 1], F32, tag="b_rs", name="b_rs")
                    nc.vector.reciprocal(rs[:], sd[:])
                    a_ = wpool.tile([128, 1], F32, tag="b_a", name="b_a")
                    nc.vector.tensor_mul(a_[:], rs[:], g_t[:, c:c + 1])
                    sh = wpool.tile([128, 1], F32, tag="b_sh", name="b_sh")
                    nc.vector.scalar_tensor_tensor(out=sh[:], in0=mv[:, 0:1], scalar=-1.0,
                                                   in1=a_[:], op0=ALU.mult, op1=ALU.mult)
                    nc.vector.tensor_add(sh[:], sh[:], be_t[:, c:c + 1])
                    nc.scalar.activation(out=out_sb[:, 32 * c:32 * c + 32], in_=hc[:],
                                         func=AF.Relu, scale=a_[:, 0:1], bias=sh[:, 0:1])

            with nc.named_scope("mlp6"):
                h6 = bigpool.tile([128, 128], F16, tag="h6sb", name="h6sb")
                mlp_layer(6, act6, h6)
                nc.sync.dma_start(out=h6_loc.ap(), in_=h6[:])
                ag(h6_loc.ap(), h6_all.ap())
            with nc.named_scope("mlp7"):
                act7 = bigpool.tile([128, 1024], F16, tag="act7", name="act7")
                for r in range(8):
                    eng = nc.scalar if r % 2 else nc.sync
                    eng.dma_start(out=act7[:, 128 * r:128 * (r + 1)],
                                  in_=h6_all[128 * r:128 * (r + 1), :])
                h7 = bigpool.tile([128, 128], F16, tag="h7sb", name="h7sb")
                mlp_layer(7, act7, h7)
                nc.sync.dma_start(out=h7_loc.ap(), in_=h7[:])
                ag(h7_loc.ap(), h7_all.ap())
            with nc.named_scope("mlp8"):
                act8 = bigpool.tile([128, 1024], F16, tag="act8", name="act8")
                for r in range(8):
                    eng = nc.scalar if r % 2 else nc.sync
                    eng.dma_start(out=act8[:, 128 * r:128 * (r + 1)],
                                  in_=h7_all[128 * r:128 * (r + 1), :])
                h8 = bigpool.tile([128, 128], F16, tag="h8sb", name="h8sb")
                mlp_layer(8, act8, h8)

            with nc.named_scope("mlp9"):
                w9t = cpool.tile([128, 512], F16, tag="w9t", name="w9t")
                nc.sync.dma_start(out=w9t[:], in_=ein["w9"][:, :])
                acc9 = apool.tile([128, 512], F32, tag="acc", name="acc9")
                for c in range(4):
                    nc.tensor.matmul(out=acc9[:32, :128], lhsT=h8[:, 32 * c:32 * c + 32],
                                     rhs=w9t[:, 128 * c:128 * (c + 1)],
                                     start=(c == 0), stop=(c == 3))
                p9sb = wpool.tile([32, 128], F32, tag="p9sb", name="p9sb")
                nc.scalar.activation(out=p9sb[:], in_=acc9[:32, :128], func=AF.Copy)
                nc.sync.dma_start(out=p9_loc.ap(), in_=p9sb[:])
                nc.gpsimd.collective_compute(
                    "AllReduce", ALU.add, replica_groups=RG,
                    ins=[p9_loc.ap().opt()], outs=[p9_red.ap().opt()])
                tot = wpool.tile([32, 128], F32, tag="f_tot", name="f_tot")
                nc.sync.dma_start(out=tot[:], in_=p9_red[:, :])
                totT = wpool.tile([128, 32], F32, tag="f_totT", name="f_totT")
                pst = ppool.tile([128, 512], F32, tag="ps", name="pst")
                identf = cpool.tile([32, 32], F32, tag="identf", name="identf")
                nc.scalar.activation(out=identf[:], in_=ident[:32, :32], func=AF.Copy)
                nc.tensor.transpose(out=pst[:128, :32], in_=tot[:], identity=identf[:])
                nc.scalar.activation(out=totT[:], in_=pst[:128, :32], func=AF.Copy)
                st6 = wpool.tile([128, 6], F32, tag="f_st6", name="f_st6")
                nc.vector.bn_stats(out=st6[:], in_=totT[:])
                mv = wpool.tile([128, 2], F32, tag="f_mv", name="f_mv")
                nc.vector.bn_aggr(out=mv[:], in_=st6[:])
                mu_ = mv[:, 0:1]
                sdf = wpool.tile([128, 1], F32, tag="f_sd", name="f_sd")
                nc.scalar.activation(out=sdf[:], in_=mv[:, 1:2], func=AF.Sqrt, bias=eps_t[:, 0:1])
                rs = wpool.tile([128, 1], F32, tag="f_rs", name="f_rs")
                nc.vector.reciprocal(rs[:], sdf[:])
                neg = wpool.tile([128, 1], F32, tag="f_neg", name="f_neg")
                nc.vector.scalar_tensor_tensor(out=neg[:], in0=mu_, scalar=-1.0,
                                               in1=rs[:], op0=ALU.mult, op1=ALU.mult)
                outt = wpool.tile([128, 32], F32, tag="f_out", name="f_out")
                nc.scalar.activation(out=outt[:], in_=totT[:], func=AF.Identity,
                                     scale=rs[:, 0:1], bias=neg[:, 0:1])
                nc.sync.dma_start(out=out_mu[:, :], in_=outt[:])

    nc.compile()
    return nc


# ---------------------------------------------------------------- entry point
def kernel(**inputs) -> np.ndarray:
    per_core, meta = _host_prep(inputs)
    if "prog" not in _CACHE:
        _CACHE["prog"] = _build_nc(meta, per_core[0])
    nc = _CACHE["prog"]
    res = bass_utils.run_bass_kernel_spmd(nc, per_core, core_ids=list(range(NCORES)))
    return np.ascontiguousarray(res.results[0]["mu"].T)


# revision 41
# speedup vs baseline: 1.1273x; 1.1273x over previous
"""Trainium2 Bass kernel for nn_Encoder_base (5x ChebConv GNN + pool + MLP).

Distribution over 8 NeuronCores (all matmuls fp16, fp32 PSUM):
  - level 0: the two props the einsum needs (Tx1[l0] = S0[l0]@X0 and
    p2t = S0[l0]@S0@X0) are composed on the HOST into single operators on
    the input X0 (2-hop edge expansion M0 = S0[l0]*S0). Edge-major X0 rows
    are pregathered host-side -> the props are pure streaming selection
    matmuls: zero indirect DMA, zero full-graph AllGather.
  - level 1: stacked dense operator T = [P_l1; S1[l1]; M1=S1[l1]*S1]
    (3072 x 4096) applied to z1, dest-sharded (128 l1-positions/core, all
    1024 batch-features wide); einsum is dest-sharded too. Comm: one
    window-chunked z1 AllGather (overlapped with the l0 pipeline) + one
    z2 AllToAll.
  - levels 2-3: batch-sharded (4 batches/core), dense-S matmuls,
    block-diagonal channel mixes in feature-major layout.
  - MLP: output-feature sharded (512 cols of W6/7/8, 512 rows of W9 per
    core); activations [128k,32] are the stationary lhsT, W streams as rhs;
    BatchNorm per-feature after a PE transpose; activations AllGathered.
"""
import numpy as np
import concourse.bass as bass
import concourse.bacc as bacc
import concourse.tile as tile
from concourse import mybir, bass_utils

F32 = mybir.dt.float32
F16 = mybir.dt.float16
I16 = mybir.dt.int16
AF = mybir.ActivationFunctionType
ALU = mybir.AluOpType
AX = mybir.AxisListType
RG = [list(range(8))]
NCORES = 8
N0, N1, N2, N3 = 16384, 4096, 1024, 128
EPS = 1e-5
H16 = np.float16

_CACHE = {}


# ---------------------------------------------------------------- host prep
def _prep_prop(row, col, we, n_dest, n_shard):
    """Sorted-by-dest edges -> 128-dest windows, 128-edge chunks, padded so
    chunk counts per window match across shards (one SPMD program).
    Emits per-chunk selection matrices sel[chunk, edge_local, dst_local]."""
    window = 128
    order = np.argsort(row, kind="stable")
    row, col, we = row[order], col[order], we[order]
    per = n_dest // n_shard
    nwin = per // window
    counts = np.zeros((n_shard, nwin), np.int64)
    lists = {}
    for s in range(n_shard):
        lo = s * per
        for wi in range(nwin):
            wlo = lo + wi * window
            a = np.searchsorted(row, wlo, side="left")
            b = np.searchsorted(row, wlo + window, side="left")
            lists[(s, wi)] = (row[a:b] - wlo, col[a:b], we[a:b])
            counts[s, wi] = (b - a + 127) // 128
    ncw = np.maximum(counts.max(axis=0), 1)
    C = int(ncw.sum())
    src = np.zeros((n_shard, C, 128), np.int64)
    dst = np.full((n_shard, C, 128), 200.0, np.float32)
    wea = np.zeros((n_shard, C, 128), np.float32)
    for s in range(n_shard):
        base = 0
        for wi in range(nwin):
            dl, cl, wl = lists[(s, wi)]
            n = len(dl)
            k = int(ncw[wi])
            src[s, base:base + k].reshape(-1)[:n] = cl
            ch = base + np.arange(n) // 128
            ep = np.arange(n) % 128
            dst[s, ch, ep] = dl
            wea[s, ch, ep] = wl
            base += k
    return [int(x) for x in ncw], src, dst, wea


def _edge_we(e, n):
    row, col = np.asarray(e[0], np.int64), np.asarray(e[1], np.int64)
    deg = np.bincount(row, minlength=n).astype(np.float32)
    dis = np.where(deg > 0, 1.0 / np.sqrt(np.maximum(deg, 1.0)), 0.0).astype(np.float32)
    return row, col, -(dis[row] * dis[col]).astype(np.float32)


def _sub_edges(row, col, we, pool_idx):
    order = np.argsort(row, kind="stable")
    row, col, we = row[order], col[order], we[order]
    starts = np.searchsorted(row, pool_idx, side="left")
    ends = np.searchsorted(row, pool_idx, side="right")
    nr, ncl, nw = [], [], []
    for i in range(len(pool_idx)):
        s, e = starts[i], ends[i]
        if e > s:
            nr.append(np.full(e - s, i, np.int64))
            ncl.append(col[s:e])
            nw.append(we[s:e])
    return np.concatenate(nr), np.concatenate(ncl), np.concatenate(nw)


def _twohop(ri, ci, wi, row, col, we, n):
    """(i,j,w1) sub-edges composed with full edges (j->k,w2): (i,k,w1*w2)."""
    order = np.argsort(row, kind="stable")
    rs, cs, ws = row[order], col[order], we[order]
    starts = np.searchsorted(rs, np.arange(n), side="left")
    ends = np.searchsorted(rs, np.arange(n), side="right")
    cnt = (ends - starts)[ci]
    I = np.repeat(ri, cnt)
    W1 = np.repeat(wi, cnt)
    base = np.repeat(starts[ci], cnt)
    within = np.arange(cnt.sum()) - np.repeat(np.cumsum(cnt) - cnt, cnt)
    offs = base + within
    return I, cs[offs], W1 * ws[offs]


def _dense_s(row, col, we, n, m):
    s = np.zeros((n, m), np.float32)
    np.add.at(s, (row, col), we)
    return s


def _tile_w(w, pack):
    """[K, M] -> [K//(128*pack) * 128, pack*M]: pack K-blocks side by side."""
    k, m = w.shape
    nb = k // 128
    t = w.reshape(nb // pack, pack, 128, m).transpose(0, 2, 1, 3)
    return np.ascontiguousarray(t.reshape((nb // pack) * 128, pack * m))


def _idx_tile(flat):
    """flat int idx list -> [128, len//16] int16 (16-part wrap, x8 replicas)."""
    return np.ascontiguousarray(
        np.tile(flat.astype(np.int16).reshape(-1, 16).T, (8, 1)))


def _chunk_tile(arr3):
    """[C, 128, W] -> [128, C*W] (chunk c at cols c*W..)."""
    C, _, W = arr3.shape
    return np.ascontiguousarray(
        arr3.transpose(1, 0, 2).reshape(128, C * W)).astype(H16)


def _host_prep(inputs):
    d = {k: np.asarray(v) for k, v in inputs.items()}
    x = d["x"].astype(np.float32)
    l0 = np.asarray(d["l0"], np.int64)
    l1 = np.asarray(d["l1"], np.int64)
    l2 = np.asarray(d["l2"], np.int64)

    X0 = np.ascontiguousarray(x.transpose(1, 0, 2).reshape(N0, 96))
    X0p = np.zeros((N0, 128), np.float32)
    X0p[:, :96] = X0
    X0l0T = np.ascontiguousarray(X0[l0].T)  # [96, 4096]

    # level-0 operators on X0: a = S0[l0] (tap1), m = S0[l0]@S0 (tap2)
    r0, c0, w0 = _edge_we(d["e0"], N0)
    r0s, c0s, w0s = _sub_edges(r0, c0, w0, l0)
    ncw_a, src_a, dst_a, we_a = _prep_prop(r0s, c0s, w0s, N1, NCORES)
    mI, mK, mW = _twohop(r0s, c0s, w0s, r0, c0, w0, N0)
    ncw_m, src_m, dst_m, we_m = _prep_prop(mI, mK, mW, N1, NCORES)

    # level-1 stacked operator T = [P_l1; S1[l1]; M1]
    r1, c1, w1 = _edge_we(d["e1"], N1)
    S1 = _dense_s(r1, c1, w1, N1, N1)
    r1s, c1s, w1s = _sub_edges(r1, c1, w1, l1)
    S1l1 = _dense_s(r1s, c1s, w1s, N2, N1)    # [1024, 4096]
    M1 = S1l1 @ S1                            # [1024, 4096]
    P_l1 = np.zeros((N2, N1), np.float32)
    P_l1[np.arange(N2), l1] = 1.0
    Tblocks = [P_l1, S1l1, M1]

    r2, c2, w2 = _edge_we(d["e2"], N2)
    S2 = _dense_s(r2, c2, w2, N2, N2)
    S2T = _tile_w(np.ascontiguousarray(S2.T), 8).astype(H16)       # [128, 8192]
    S2l2T = _tile_w(np.ascontiguousarray(S2[l2].T), 8).astype(H16)  # [128, 1024]
    P_l2 = np.zeros((N2, 128), np.float32)
    P_l2[l2, np.arange(128)] = 1.0
    P_l2 = _tile_w(P_l2, 8).astype(H16)                             # [128, 1024]

    r3, c3, w3 = _edge_we(d["e3"], N3)
    S3T = np.ascontiguousarray(_dense_s(r3, c3, w3, N3, N3).T).astype(H16)

    def wmod(W):
        return W[0] - W[2], W[1], 2.0 * W[2]

    Wm1 = wmod(d["Wc1"].astype(np.float32))
    Wm = [wmod(d[f"Wc{i}"].astype(np.float32)) for i in (2, 3, 4, 5)]
    eye4 = np.eye(4, dtype=np.float32)

    per_core = []
    for k in range(NCORES):
        m = {}
        m["identbf"] = np.eye(128, dtype=np.float32).astype(H16)
        m["iota"] = np.tile(np.arange(128, dtype=np.float32), (128, 1))
        m["epsv"] = np.full((128, 1), EPS, np.float32)
        m["X0l0Tw"] = np.ascontiguousarray(
            X0l0T[:, 512 * k:512 * (k + 1)]).astype(H16)
        m["Xg_a"] = _chunk_tile(X0p[src_a[k]].astype(H16))
        m["a_dst"] = np.ascontiguousarray(dst_a[k].T)
        m["a_we"] = np.ascontiguousarray(we_a[k].T)
        m["Xg_m"] = _chunk_tile(X0p[src_m[k]].astype(H16))
        m["m_dst"] = np.ascontiguousarray(dst_m[k].T)
        m["m_we"] = np.ascontiguousarray(we_m[k].T)
        # stacked-T lhsT chunks: block b, k-chunk kk at cols (b*32+kk)*128
        tt = np.zeros((128, 96 * 128), np.float32)
        for b, blk in enumerate(Tblocks):
            bt = blk[128 * k:128 * (k + 1), :].T  # [4096, 128]
            for kk in range(32):
                tt[:, (b * 32 + kk) * 128:(b * 32 + kk + 1) * 128] = \
                    bt[128 * kk:128 * (kk + 1), :]
        m["Tt"] = tt.astype(H16)
        m["S2T"] = S2T
        m["S2l2T"] = S2l2T
        m["P_l2"] = P_l2
        m["S3T"] = S3T
        for g in range(8):
            for t in range(3):
                bw = np.zeros((96, 128), np.float32)
                for j in range(4):
                    bg = 4 * g + j
                    bw[3 * bg:3 * bg + 3, 32 * j:32 * j + 32] = Wm1[t]
                m[f"bigw0_{g}_{t}"] = bw.astype(H16)
        for lev in range(4):
            for t in range(3):
                m[f"bigw{lev + 1}_{t}"] = np.kron(eye4, Wm[lev][t]).astype(H16)
        for lev, nm in ((1, "b1"), (2, "b2"), (3, "b3"), (4, "b4"), (5, "b5")):
            m[f"bias{lev}"] = np.tile(d[nm].astype(np.float32), 4).reshape(128, 1)
        for li in (6, 7, 8):
            W = d[f"W{li}"].astype(np.float32)[:, 512 * k:512 * k + 512]
            m[f"w{li}"] = np.ascontiguousarray(
                W.reshape(32, 128, 512).transpose(1, 0, 2).reshape(128, 32 * 512)
            ).astype(H16)
            m[f"g{li}"] = np.ascontiguousarray(
                d[f"g{li}"].astype(np.float32)[512 * k:512 * k + 512].reshape(4, 128).T)
            m[f"be{li}"] = np.ascontiguousarray(
                d[f"be{li}"].astype(np.float32)[512 * k:512 * k + 512].reshape(4, 128).T)
        W9 = d["W9"].astype(np.float32)[512 * k:512 * k + 512]  # [512, 128]
        m["w9"] = np.ascontiguousarray(
            W9.reshape(4, 128, 128).transpose(1, 0, 2).reshape(128, 512)).astype(H16)
        per_core.append(m)

    meta = {"a": ncw_a, "m": ncw_m}
    return per_core, meta


# ---------------------------------------------------------------- device program
def _build_nc(meta, shapes):
    nc = bacc.Bacc("TRN2", target_bir_lowering=False, debug=False, num_devices=NCORES)
    ein = {}
    for name, arr in shapes.items():
        dt = {np.dtype(np.int16): I16, np.dtype(H16): F16,
              np.dtype(np.float32): F32}[arr.dtype]
        ein[name] = nc.dram_tensor(name, list(arr.shape), dt, kind="ExternalInput")
    out_mu = nc.dram_tensor("mu", [128, 32], F32, kind="ExternalOutput")

    warm_all = nc.dram_tensor("warm_all", [8, 8], F32, addr_space="Shared")
    warm_loc = nc.dram_tensor("warm_loc", [1, 8], F32)
    z1c = [nc.dram_tensor(f"z1c_{i}", [128, 1024], F16) for i in range(4)]
    z1ag = [nc.dram_tensor(f"z1ag_{i}", [1024, 1024], F16, addr_space="Shared")
            for i in range(4)]
    z2T_loc = nc.dram_tensor("z2T_loc", [1024, 128], F16)
    z2a2a = nc.dram_tensor("z2a2a", [1024, 128], F16)
    x_loc = nc.dram_tensor("x_loc", [4, 4096], F16)
    x_all = nc.dram_tensor("x_all", [32, 4096], F16, addr_space="Shared")
    h6_loc = nc.dram_tensor("h6_loc", [128, 128], F16)
    h6_all = nc.dram_tensor("h6_all", [1024, 128], F16, addr_space="Shared")
    h7_loc = nc.dram_tensor("h7_loc", [128, 128], F16)
    h7_all = nc.dram_tensor("h7_all", [1024, 128], F16, addr_space="Shared")
    p9_loc = nc.dram_tensor("p9_loc", [32, 128], F32)
    p9_red = nc.dram_tensor("p9_red", [32, 128], F32, addr_space="Shared")

    def ag(loc_ap, all_ap):
        nc.gpsimd.collective_compute(
            "AllGather", ALU.bypass, replica_groups=RG,
            ins=[loc_ap.opt()], outs=[all_ap.opt()])

    with tile.TileContext(nc) as tc:
        with (
            tc.tile_pool(name="const", bufs=1) as cpool,
            tc.tile_pool(name="big", bufs=1) as bigpool,
            tc.tile_pool(name="work", bufs=3) as wpool,
            tc.tile_pool(name="wload", bufs=2) as wlpool,
            tc.tile_pool(name="psA", bufs=3, space="PSUM") as ppool,
            tc.tile_pool(name="psT", bufs=2, space="PSUM") as tpool,
            tc.tile_pool(name="psB", bufs=1, space="PSUM") as apool,
        ):
            ident = cpool.tile([128, 128], F16, tag="identbf", name="identbf")
            nc.sync.dma_start(out=ident[:], in_=ein["identbf"][:, :])
            iota_t = cpool.tile([128, 128], F32, tag="iota", name="iota")
            nc.sync.dma_start(out=iota_t[:], in_=ein["iota"][:, :])
            eps_t = cpool.tile([128, 1], F32, tag="epsv", name="epsv")
            nc.sync.dma_start(out=eps_t[:], in_=ein["epsv"][:, :])

            def load_const(name, dt=F16):
                t = cpool.tile(list(shapes[name].shape), dt, tag=name)
                nc.sync.dma_start(out=t[:], in_=ein[name][:, :])
                return t

            GRP = 16

            def grp_load(pref, g0, gc, tag, eng=None, grp=None):
                sl = wpool.tile([128, (grp or GRP) * 128], F16, tag=tag,
                                name=tag, bufs=2)
                (eng or nc.sync).dma_start(out=sl[:, :gc * 128],
                                           in_=ein[pref][:, g0 * 128:(g0 + gc) * 128])
                return sl

            def transp(src_ap, dst_ap):
                p, f = src_ap.shape
                ps = tpool.tile([128, 128], F16, tag="tp", name="tp")
                nc.tensor.transpose(out=ps[:f, :p], in_=src_ap, identity=ident[:p, :p])
                nc.scalar.activation(out=dst_ap, in_=ps[:f, :p], func=AF.Copy)

            def einsum_win(bigw, taps, Din, width, out_ap, func, bias_ap):
                ps = ppool.tile([128, 512], F32, tag="ps", name="ps")
                for t in range(3):
                    nc.tensor.matmul(out=ps[:, :width], lhsT=bigw[t][:Din, :],
                                     rhs=taps[t], start=(t == 0), stop=(t == 2))
                f2 = AF.Identity if func == AF.Copy else func
                nc.scalar.activation(out=out_ap, in_=ps[:, :width], func=f2, bias=bias_ap)

            # warm up the CC ring while level-0 computes
            with nc.named_scope("warmup"):
                ag(warm_loc.ap(), warm_all.ap())

            # ====== LEVEL 0: per-window pipeline of props -> einsum -> AG ====
            # tap1 = S0[l0]@X0, tap2 = (S0[l0]@S0)@X0, then the channel-mix
            # einsum for window wi immediately, then AllGather that window.
            with nc.named_scope("l0"):
                Ca, Cm = sum(meta["a"]), sum(meta["m"])
                cura = {"g0": -1}
                curm = {"g0": -1}

                def get_grp(cur, cc, C, xg, xtag, grp):
                    g0 = (cc // grp) * grp
                    if g0 != cur["g0"]:
                        gc = min(grp, C - g0)
                        cur["g0"] = g0
                        cur["x"] = grp_load(xg, g0, gc, xtag, eng=nc.scalar, grp=grp)
                    return cur, (cc - cur["g0"]) * 128

                def mk_sel(dw_t, cc):
                    sel = wpool.tile([128, 128], F16, tag="sel", name="sel", bufs=4)
                    nc.vector.tensor_scalar(
                        out=sel[:], in0=iota_t[:], scalar1=dw_t[0][:, cc:cc + 1],
                        scalar2=dw_t[1][:, cc:cc + 1], op0=ALU.is_equal, op1=ALU.mult)
                    return sel

                adw = [load_const("a_dst", F32), load_const("a_we", F32)]
                mdw = [load_const("m_dst", F32), load_const("m_we", F32)]
                tt = cpool.tile([128, 96 * 128], F16, tag="Tt", name="Tt")
                nc.sync.dma_start(out=tt[:, :6144], in_=ein["Tt"][:, :6144])
                nc.sync.dma_start(out=tt[:, 6144:], in_=ein["Tt"][:, 6144:])
                accs = [bigpool.tile([128, 1024], F16, tag=f"accT{b}", name=f"accT{b}")
                        for b in range(3)]

                def t_part(part):
                    for h in range(2):
                        z1sc = wpool.tile([128, 4096], F16, tag="z1sc",
                                          name="z1sc", bufs=2)
                        nc.sync.dma_start(
                            out=z1sc[:].rearrange("p (q d) -> p q d", d=1024),
                            in_=z1ag[part].ap()[512 * h:512 * (h + 1)]
                                .rearrange("(q p) d -> p q d", p=128))
                        for b in range(3):
                            psa = ppool.tile([128, 512], F32, tag="ps", name="ps")
                            psb = ppool.tile([128, 512], F32, tag="ps", name="ps")
                            for q in range(4):
                                kk = 4 * (4 * h + q) + part
                                lh = tt[:, (b * 32 + kk) * 128:(b * 32 + kk + 1) * 128]
                                nc.tensor.matmul(out=psa[:, :512], lhsT=lh,
                                                 rhs=z1sc[:, q * 1024:q * 1024 + 512],
                                                 start=(q == 0), stop=(q == 3))
                                nc.tensor.matmul(out=psb[:, :512], lhsT=lh,
                                                 rhs=z1sc[:, q * 1024 + 512:(q + 1) * 1024],
                                                 start=(q == 0), stop=(q == 3))
                            if part == 0 and h == 0:
                                nc.scalar.activation(out=accs[b][:, :512],
                                                     in_=psa[:, :512], func=AF.Copy)
                                nc.scalar.activation(out=accs[b][:, 512:],
                                                     in_=psb[:, :512], func=AF.Copy)
                            else:
                                nc.vector.tensor_add(accs[b][:, :512], accs[b][:, :512],
                                                     psa[:, :512])
                                nc.vector.tensor_add(accs[b][:, 512:], accs[b][:, 512:],
                                                     psb[:, :512])

                bias1 = load_const("bias1", F32)
                x0w = cpool.tile([96, 512], F16, tag="X0l0Tw", name="X0l0Tw")
                nc.sync.dma_start(out=x0w[:], in_=ein["X0l0Tw"][:, :])
                bw0g = [[load_const(f"bigw0_{g}_{t}") for t in range(3)]
                        for g in range(8)]
                tap1_sb = bigpool.tile([96, 512], F16, tag="tap1_sb", name="tap1_sb")
                tap2_sb = bigpool.tile([96, 512], F16, tag="tap2_sb", name="tap2_sb")
                ba, bm = 0, 0
                for wi in range(4):
                    psa = ppool.tile([128, 512], F32, tag="ps", name="ps")
                    for c in range(meta["a"][wi]):
                        g, lo = get_grp(cura, ba + c, Ca, "Xg_a", "xga", 8)
                        nc.tensor.matmul(out=psa[:96, :128], lhsT=g["x"][:, lo:lo + 96],
                                         rhs=mk_sel(adw, ba + c)[:],
                                         start=(c == 0), stop=(c == meta["a"][wi] - 1))
                    nc.scalar.activation(out=tap1_sb[:, wi * 128:(wi + 1) * 128],
                                         in_=psa[:96, :128], func=AF.Copy)
                    ba += meta["a"][wi]
                    psm = ppool.tile([128, 512], F32, tag="ps", name="ps")
                    for c in range(meta["m"][wi]):
                        g, lo = get_grp(curm, bm + c, Cm, "Xg_m", "xgm", 16)
                        nc.tensor.matmul(out=psm[:96, :128], lhsT=g["x"][:, lo:lo + 96],
                                         rhs=mk_sel(mdw, bm + c)[:],
                                         start=(c == 0), stop=(c == meta["m"][wi] - 1))
                    nc.scalar.activation(out=tap2_sb[:, wi * 128:(wi + 1) * 128],
                                         in_=psm[:96, :128], func=AF.Copy)
                    bm += meta["m"][wi]
                    for g in range(8):
                        z1gT = wpool.tile([128, 128], F16, tag="z1Tw", name="z1Tw")
                        einsum_win(bw0g[g],
                                   [x0w[:, wi * 128:(wi + 1) * 128],
                                    tap1_sb[:96, wi * 128:(wi + 1) * 128],
                                    tap2_sb[:96, wi * 128:(wi + 1) * 128]],
                                   96, 128, z1gT[:], AF.Copy, bias1[:, 0:1])
                        t = wpool.tile([128, 128], F16, tag="z1nc", name="z1nc")
                        transp(z1gT[:], t[:])
                        nc.sync.dma_start(
                            out=z1c[wi][:, 128 * g:128 * (g + 1)], in_=t[:])
                    with nc.named_scope(f"agz1_{wi}"):
                        ag(z1c[wi].ap(), z1ag[wi].ap())
                with nc.named_scope("l1_T"):
                    for part in range(4):
                        t_part(part)

            # ============ LEVEL 1 einsum (dest-sharded) -> z2T ===============
            with nc.named_scope("l1_einsum"):
                bw1 = [load_const(f"bigw1_{t}") for t in range(3)]
                bias2 = load_const("bias2", F32)
                tapTs = []
                for b in range(3):
                    tapT = bigpool.tile([128, 1024], F16, tag=f"tapT{b}", name=f"tapT{b}")
                    for f in range(8):
                        transp(accs[b][:, 128 * f:128 * (f + 1)],
                               tapT[:, 128 * f:128 * (f + 1)])
                    tapTs.append(tapT)
                for fg in range(8):
                    z2fg = wpool.tile([128, 128], F16, tag="z2fg", name="z2fg")
                    einsum_win(bw1, [tapTs[0][:, 128 * fg:128 * (fg + 1)],
                                     tapTs[1][:, 128 * fg:128 * (fg + 1)],
                                     tapTs[2][:, 128 * fg:128 * (fg + 1)]],
                               128, 128, z2fg[:], AF.Tanh, bias2[:, 0:1])
                    nc.sync.dma_start(out=z2T_loc[128 * fg:128 * (fg + 1), :],
                                      in_=z2fg[:])
            with nc.named_scope("a2a_z2"):
                nc.gpsimd.collective_compute(
                    "AllToAll", ALU.bypass, replica_groups=RG,
                    ins=[z2T_loc.ap().opt()], outs=[z2a2a.ap().opt()])

            # ====== z2n assembly (batch-sharded node-major) ==================
            z2n = bigpool.tile([128, 8 * 128], F16, tag="z2n", name="z2n")
            with nc.named_scope("z2n_asm"):
                zb = wpool.tile([128, 1024], F16, tag="zb", name="zb", bufs=1)
                for ci in range(8):
                    nc.sync.dma_start(out=zb[:, ci * 128:(ci + 1) * 128],
                                      in_=z2a2a[128 * ci:128 * (ci + 1), :])
                for ci in range(8):
                    transp(zb[:, ci * 128:(ci + 1) * 128],
                           z2n[:, ci * 128:(ci + 1) * 128])

            # ================= LEVEL 2 (dense) =================
            with nc.named_scope("l2"):
                t1_l2 = bigpool.tile([128, 8 * 128], F16, tag="t1_l2", name="t1_l2")
                for half in range(2):
                    s2t = wlpool.tile([128, 4096], F16, tag="wld", name="wld")
                    nc.sync.dma_start(out=s2t[:], in_=ein["S2T"][:, 4096 * half:4096 * (half + 1)])
                    for dc in range(8):
                        ps = ppool.tile([128, 512], F32, tag="ps", name="ps")
                        for kk in range(4):
                            kc = half * 4 + kk
                            nc.tensor.matmul(
                                out=ps[:, :128],
                                lhsT=s2t[:, kk * 1024 + dc * 128: kk * 1024 + dc * 128 + 128],
                                rhs=z2n[:, kc * 128:(kc + 1) * 128],
                                start=(kk == 0), stop=(kk == 3))
                        if half == 0:
                            nc.scalar.activation(out=t1_l2[:, dc * 128:(dc + 1) * 128],
                                                 in_=ps[:, :128], func=AF.Copy)
                        else:
                            nc.vector.tensor_add(t1_l2[:, dc * 128:(dc + 1) * 128],
                                                 t1_l2[:, dc * 128:(dc + 1) * 128],
                                                 ps[:, :128])
                s2l2 = cpool.tile([128, 1024], F16, tag="s2l2", name="s2l2")
                nc.sync.dma_start(out=s2l2[:], in_=ein["S2l2T"][:, :])
                ps = ppool.tile([128, 512], F32, tag="ps", name="ps")
                for kc in range(8):
                    nc.tensor.matmul(out=ps[:, :128], lhsT=s2l2[:, kc * 128:(kc + 1) * 128],
                                     rhs=t1_l2[:, kc * 128:(kc + 1) * 128],
                                     start=(kc == 0), stop=(kc == 7))
                p2n_l2 = wpool.tile([128, 128], F16, tag="p2n_l2", name="p2n_l2")
                nc.scalar.activation(out=p2n_l2[:], in_=ps[:, :128], func=AF.Copy)
                pl2 = cpool.tile([128, 1024], F16, tag="pl2", name="pl2")
                nc.sync.dma_start(out=pl2[:], in_=ein["P_l2"][:, :])
                z2l2T = wpool.tile([128, 128], F16, tag="z2l2T", name="z2l2T")
                psg = ppool.tile([128, 512], F32, tag="ps", name="ps")
                for kc in range(8):
                    nc.tensor.matmul(out=psg[:, :128], lhsT=z2n[:, kc * 128:(kc + 1) * 128],
                                     rhs=pl2[:, kc * 128:(kc + 1) * 128],
                                     start=(kc == 0), stop=(kc == 7))
                nc.scalar.activation(out=z2l2T[:], in_=psg[:, :128], func=AF.Copy)
                t1l2T = wpool.tile([128, 128], F16, tag="t1l2T", name="t1l2T")
                psg2 = ppool.tile([128, 512], F32, tag="ps", name="ps")
                for kc in range(8):
                    nc.tensor.matmul(out=psg2[:, :128], lhsT=t1_l2[:, kc * 128:(kc + 1) * 128],
                                     rhs=pl2[:, kc * 128:(kc + 1) * 128],
                                     start=(kc == 0), stop=(kc == 7))
                nc.scalar.activation(out=t1l2T[:], in_=psg2[:, :128], func=AF.Copy)
                p2l2T = wpool.tile([128, 128], F16, tag="p2l2T", name="p2l2T")
                transp(p2n_l2[:], p2l2T[:])
                bw2 = [load_const(f"bigw2_{t}") for t in range(3)]
                bias3 = load_const("bias3", F32)
                z3T = wpool.tile([128, 128], F16, tag="z3T", name="z3T")
                einsum_win(bw2, [z2l2T[:], t1l2T[:], p2l2T[:]], 128, 128,
                           z3T[:], AF.Tanh, bias3[:, 0:1])
                z3n = wpool.tile([128, 128], F16, tag="z3n", name="z3n")
                transp(z3T[:], z3n[:])

            # ================= LEVEL 3 =================
            with nc.named_scope("l3"):
                s3t = cpool.tile([128, 128], F16, tag="s3t", name="s3t")
                nc.sync.dma_start(out=s3t[:], in_=ein["S3T"][:, :])
                bias4 = load_const("bias4", F32)
                bias5 = load_const("bias5", F32)

                def conv_l3(zn, zT, bw_pref, bias_t, func, keep):
                    t1T = wpool.tile([128, 128], F16, tag=keep + "t1T", name=keep + "t1T")
                    ps = ppool.tile([128, 512], F32, tag="ps", name="ps")
                    nc.tensor.matmul(out=ps[:, :128], lhsT=zn, rhs=s3t[:], start=True, stop=True)
                    nc.scalar.activation(out=t1T[:], in_=ps[:, :128], func=AF.Copy)
                    t1n_ = wpool.tile([128, 128], F16, tag=keep + "t1n", name=keep + "t1n")
                    transp(t1T[:], t1n_[:])
                    p2T_ = wpool.tile([128, 128], F16, tag=keep + "p2T", name=keep + "p2T")
                    ps2 = ppool.tile([128, 512], F32, tag="ps", name="ps")
                    nc.tensor.matmul(out=ps2[:, :128], lhsT=t1n_[:], rhs=s3t[:], start=True, stop=True)
                    nc.scalar.activation(out=p2T_[:], in_=ps2[:, :128], func=AF.Copy)
                    bw = [load_const(f"{bw_pref}_{t}") for t in range(3)]
                    outT = wpool.tile([128, 128], F16, tag=keep + "oT", name=keep + "oT")
                    einsum_win(bw, [zT, t1T[:], p2T_[:]], 128, 128, outT[:], func, bias_t[:, 0:1])
                    outn = wpool.tile([128, 128], F16, tag=keep + "on", name=keep + "on")
                    transp(outT[:], outn[:])
                    return outn, outT

                z4n, z4T = conv_l3(z3n[:], z3T[:], "bigw3", bias4, AF.Tanh, "c4")
                o5n, o5T = conv_l3(z4n[:], z4T[:], "bigw4", bias5, AF.Copy, "c5")

            # ================= MLP input assembly =================
            with nc.named_scope("mlp_in"):
                nc.sync.dma_start(
                    out=x_loc.ap().rearrange("b (n c) -> n b c", c=32),
                    in_=o5n[:].rearrange("n (b c) -> n b c", c=32))
                ag(x_loc.ap(), x_all.ap())
                xT_sb = bigpool.tile([32, 4096], F16, tag="xT_sb", name="xT_sb")
                nc.sync.dma_start(out=xT_sb[:], in_=x_all[:, :])
                act6 = bigpool.tile([128, 1024], F16, tag="act6", name="act6")
                for i in range(32):
                    transp(xT_sb[:, 128 * i:128 * (i + 1)], act6[:, 32 * i:32 * i + 32])

            # ================= MLP =================
            def mlp_layer(li, act_sb, out_sb):
                g_t = load_const(f"g{li}", F32)
                be_t = load_const(f"be{li}", F32)
                wt = wlpool.tile([128, 32 * 512], F16, tag="wld", name="wld")
                nc.sync.dma_start(out=wt[:, :8192], in_=ein[f"w{li}"][:, :8192])
                nc.sync.dma_start(out=wt[:, 8192:], in_=ein[f"w{li}"][:, 8192:])
                acc = apool.tile([128, 512], F32, tag="acc", name="acc")
                for k2 in range(32):
                    nc.tensor.matmul(out=acc[:32, :], lhsT=act_sb[:, 32 * k2:32 * k2 + 32],
                                     rhs=wt[:, 512 * k2:512 * (k2 + 1)],
                                     start=(k2 == 0), stop=(k2 == 31))
                hb = wpool.tile([32, 512], F16, tag="hb", name="hb")
                nc.scalar.activation(out=hb[:], in_=acc[:32, :], func=AF.Copy)
                for c in range(4):
                    hc = wpool.tile([128, 32], F16, tag="hc", name="hc")
                    transp(hb[:, 128 * c:128 * (c + 1)], hc[:])
                    st6 = wpool.tile([128, 6], F32, tag="b_st6", name="b_st6")
                    nc.vector.bn_stats(out=st6[:], in_=hc[:])
                    mv = wpool.tile([128, 2], F32, tag="b_mv", name="b_mv")
                    nc.vector.bn_aggr(out=mv[:], in_=st6[:])
                    sd = wpool.tile([128, 1], F32, tag="b_sd", name="b_sd")
                    nc.scalar.activation(out=sd[:], in_=mv[:, 1:2], func=AF.Sqrt,
                                         bias=eps_t[:, 0:1])
                    rs = wpool.tile([128, 1], F32, tag="b_rs", name="b_rs")
                    nc.vector.reciprocal(rs[:], sd[:])
                    a_ = wpool.tile([128, 1], F32, tag="b_a", name="b_a")
                    nc.vector.tensor_mul(a_[:], rs[:], g_t[:, c:c + 1])
                    sh = wpool.tile([128, 1], F32, tag="b_sh", name="b_sh")
                    nc.vector.scalar_tensor_tensor(out=sh[:], in0=mv[:, 0:1], scalar=-1.0,
                                                   in1=a_[:], op0=ALU.mult, op1=ALU.mult)
                    nc.vector.tensor_add(sh[:], sh[:], be_t[:, c:c + 1])
                    nc.scalar.activation(out=out_sb[:, 32 * c:32 * c + 32], in_=hc[:],
                                         func=AF.Relu, scale=a_[:, 0:1], bias=sh[:, 0:1])

            with nc.named_scope("mlp6"):
                h6 = bigpool.tile([128, 128], F16, tag="h6sb", name="h6sb")
                mlp_layer(6, act6, h6)
                nc.sync.dma_start(out=h6_loc.ap(), in_=h6[:])
                ag(h6_loc.ap(), h6_all.ap())
            with nc.named_scope("mlp7"):
                act7 = bigpool.tile([128, 1024], F16, tag="act7", name="act7")
                for r in range(8):
                    nc.sync.dma_start(out=act7[:, 128 * r:128 * (r + 1)],
                                      in_=h6_all[128 * r:128 * (r + 1), :])
                h7 = bigpool.tile([128, 128], F16, tag="h7sb", name="h7sb")
                mlp_layer(7, act7, h7)
                nc.sync.dma_start(out=h7_loc.ap(), in_=h7[:])
                ag(h7_loc.ap(), h7_all.ap())
            with nc.named_scope("mlp8"):
                act8 = bigpool.tile([128, 1024], F16, tag="act8", name="act8")
                for r in range(8):
                    nc.sync.dma_start(out=act8[:, 128 * r:128 * (r + 1)],
                                      in_=h7_all[128 * r:128 * (r + 1), :])
                h8 = bigpool.tile([128, 128], F16, tag="h8sb", name="h8sb")
                mlp_layer(8, act8, h8)

            with nc.named_scope("mlp9"):
                w9t = cpool.tile([128, 512], F16, tag="w9t", name="w9t")
                nc.sync.dma_start(out=w9t[:], in_=ein["w9"][:, :])
                acc9 = apool.tile([128, 512], F32, tag="acc", name="acc9")
                for c in range(4):
                    nc.tensor.matmul(out=acc9[:32, :128], lhsT=h8[:, 32 * c:32 * c + 32],
                                     rhs=w9t[:, 128 * c:128 * (c + 1)],
                                     start=(c == 0), stop=(c == 3))
                p9sb = wpool.tile([32, 128], F32, tag="p9sb", name="p9sb")
                nc.scalar.activation(out=p9sb[:], in_=acc9[:32, :128], func=AF.Copy)
                nc.sync.dma_start(out=p9_loc.ap(), in_=p9sb[:])
                nc.gpsimd.collective_compute(
                    "AllReduce", ALU.add, replica_groups=RG,
                    ins=[p9_loc.ap().opt()], outs=[p9_red.ap().opt()])
                tot = wpool.tile([32, 128], F32, tag="f_tot", name="f_tot")
                nc.sync.dma_start(out=tot[:], in_=p9_red[:, :])
                totT = wpool.tile([128, 32], F32, tag="f_totT", name="f_totT")
                pst = ppool.tile([128, 512], F32, tag="ps", name="pst")
                identf = cpool.tile([32, 32], F32, tag="identf", name="identf")
                nc.scalar.activation(out=identf[:], in_=ident[:32, :32], func=AF.Copy)
                nc.tensor.transpose(out=pst[:128, :32], in_=tot[:], identity=identf[:])
                nc.scalar.activation(out=totT[:], in_=pst[:128, :32], func=AF.Copy)
                st6 = wpool.tile([128, 6], F32, tag="f_st6", name="f_st6")
                nc.vector.bn_stats(out=st6[:], in_=totT[:])
                mv = wpool.tile([128, 2], F32, tag="f_mv", name="f_mv")
                nc.vector.bn_aggr(out=mv[:], in_=st6[:])
                mu_ = mv[:, 0:1]
                sdf = wpool.tile([128, 1], F32, tag="f_sd", name="f_sd")
                nc.scalar.activation(out=sdf[:], in_=mv[:, 1:2], func=AF.Sqrt, bias=eps_t[:, 0:1])
                rs = wpool.tile([128, 1], F32, tag="f_rs", name="f_rs")
                nc.vector.reciprocal(rs[:], sdf[:])
                neg = wpool.tile([128, 1], F32, tag="f_neg", name="f_neg")
                nc.vector.scalar_tensor_tensor(out=neg[:], in0=mu_, scalar=-1.0,
                                               in1=rs[:], op0=ALU.mult, op1=ALU.mult)
                outt = wpool.tile([128, 32], F32, tag="f_out", name="f_out")
                nc.scalar.activation(out=outt[:], in_=totT[:], func=AF.Identity,
                                     scale=rs[:, 0:1], bias=neg[:, 0:1])
                nc.sync.dma_start(out=out_mu[:, :], in_=outt[:])

    nc.compile()
    return nc


# ---------------------------------------------------------------- entry point
def kernel(**inputs) -> np.ndarray:
    per_core, meta = _host_prep(inputs)
    if "prog" not in _CACHE:
        _CACHE["prog"] = _build_nc(meta, per_core[0])
    nc = _CACHE["prog"]
    res = bass_utils.run_bass_kernel_spmd(nc, per_core, core_ids=list(range(NCORES)))
    return np.ascontiguousarray(res.results[0]["mu"].T)
